# revision 1
# baseline (speedup 1.0000x reference)
"""Trainium2 Bass kernel for a 3x3 stride-1 pad-1 conv:
x (32,128,64,64) f32, weight (256,128,3,3) f32, bias (256,) f32
-> out (32,256,64,64) f32.

Strategy: data-parallel over batch across 8 NeuronCores (4 samples each).
Per core, the conv is 9 shifted matmuls accumulating in PSUM:
  out[co, hw] = sum_{kh,kw} W[co, :, kh, kw] @ xpad[:, h+kh, w+kw]
C_in=128 sits on the SBUF partition dim; the moving operand is a
[128, 8*64] window of the zero-padded image (rows strided by 66), and the
stationary operand is the [ci, co] transpose of one (kh,kw) weight slice.
Matmuls run as float32r (full-rate fp32 mode; 4x faster than plain fp32).
"""

import numpy as np

import concourse.bass as bass
from concourse import bacc
import concourse.mybir as mybir
import concourse.tile as tile
from concourse.bass_utils import run_bass_kernel_spmd
from concourse.masks import make_identity

N_CORES = 8
B_FULL = 32
B_LOCAL = B_FULL // N_CORES  # 4
CI = 128
CO = 256
H = W = 64
HP = WP = 66  # zero-padded image
ROWS = 8  # output rows per PSUM tile -> free dim 8*64 = 512
N_T = H // ROWS
F32 = mybir.dt.float32
F32R = mybir.dt.float32r


def build_nc():
    nc = bacc.Bacc()
    x_d = nc.dram_tensor("x", [B_LOCAL, CI, H, W], F32, kind="ExternalInput")
    w_d = nc.dram_tensor("weight", [CO, CI, 3, 3], F32, kind="ExternalInput")
    b_d = nc.dram_tensor("bias", [CO], F32, kind="ExternalInput")
    o_d = nc.dram_tensor("out", [B_LOCAL, CO, H, W], F32, kind="ExternalOutput")

    with tile.TileContext(nc) as tc:
        with (
            tc.tile_pool(name="const", bufs=1) as const,
            tc.tile_pool(name="xstage", bufs=B_LOCAL) as xstage,
            tc.tile_pool(name="xpad", bufs=B_LOCAL) as xpool,
            tc.tile_pool(name="obuf", bufs=6) as opool,
            tc.tile_pool(name="psum", bufs=6, space="PSUM") as pspool,
            tc.tile_pool(name="psum_tr", bufs=2, space="PSUM") as trpool,
        ):
            # Load weight as [co_p, cb, ci*9] (contiguous per partition), then
            # transpose each 128x128 (kh,kw,cb) slice on the PE to [ci, co_p].
            # All input loads ride the ACT HWDGE ring (qActDynamicHW): the
            # sync ring carrying the 64 output stores must stay clear, and
            # rings contend for the same SDMA engines anyway. Ring order is
            # chosen so compute never waits: w0, x0-chunk0, w1, bias,
            # x0-chunk1, x1..x3.
            ident = const.tile([128, 128], F32)
            make_identity(nc, ident)
            # PE_HAM flips the clock gate 1.2->2.4 GHz only after ~3.4us of
            # sustained PE activity; burn the unavoidable initial DMA wait on
            # dummy transposes so the real matmuls start at full clock.
            for _ in range(8):
                warm = trpool.tile([128, 128], F32, tag="tr")
                nc.tensor.transpose(warm, ident, ident)

            w_raw = const.tile([128, 2, CI * 9], F32)
            w_v = w_d.rearrange("(cb cp) ci kh kw -> cp cb (ci kh kw)", cb=2)
            nc.scalar.dma_start(w_raw[:, 0], w_v[:, 0])
            bias_sb = const.tile([128, 2], F32)

            # Prefetch ALL samples up-front: contiguous DMA into an fp32 stage,
            # then a DVE copy does padding insertion + the required fp32r
            # rounding (fp32r matmul operands must be produced by a compute op).
            # Sample 0 is split in two chunks so compute starts sooner.
            x_v = x_d.rearrange("b c h w -> b c (h w)")
            # memset cannot target f32r tiles; borders get zeroed via a
            # tensor_copy from this fp32 zero row (a valid f32r producer).
            zrow = const.tile([128, WP], F32)
            nc.vector.memset(zrow, 0.0)

            def alloc_sample():
                xin = xstage.tile([128, H * W], F32)
                xp = xpool.tile([128, HP, WP], F32R)
                nc.vector.tensor_copy(xp[:, 0, :], zrow)
                nc.vector.tensor_copy(xp[:, HP - 1, :], zrow)
                nc.vector.tensor_copy(xp[:, :, 0], zrow)
                nc.vector.tensor_copy(xp[:, :, WP - 1], zrow)
                return xin, xp

            def load_chunk(xin, xp, b, r0, rows):
                nc.scalar.dma_start(
                    xin[:, r0 * W : (r0 + rows) * W],
                    x_v[b, :, r0 * W : (r0 + rows) * W],
                )
                nc.vector.tensor_copy(
                    xp[:, 1 + r0 : 1 + r0 + rows, 1 : W + 1],
                    xin[:, r0 * W : (r0 + rows) * W].rearrange(
                        "p (h w) -> p h w", w=W
                    ),
                )

            # chunk0 = input rows 0..35: covers every padded row (<=33+) the
            # first-half tiles (t0..t3) read; chunk1 = rows 36..63.
            SPLIT = 32
            xin0, xp0 = alloc_sample()
            load_chunk(xin0, xp0, 0, 0, SPLIT)
            load_chunk(xin0, xp0, 0, SPLIT, H - SPLIT)
            nc.scalar.dma_start(w_raw[:, 1], w_v[:, 1])
            nc.scalar.dma_start(bias_sb, b_d.rearrange("(cb cp) -> cp cb", cb=2))
            xps = [xp0]

            # Emit the weight transposes BEFORE the remaining sample loads:
            # DVE serves work in emission-priority order, and the PE is
            # stalled on exactly these w_t copies at startup — sample 1-3
            # CASTs are not needed for another ~30us.
            w_t = const.tile([128, 18, 128], F32R)  # [ci, cb*9+k, co_p]
            for cb in range(2):
                w_cb = w_raw[:, cb, :].rearrange("p (ci k) -> p k ci", k=9)
                for k in range(9):
                    ptr = trpool.tile([128, 128], F32, tag="tr")
                    nc.tensor.transpose(ptr, w_cb[:, k, :], ident)
                    nc.vector.tensor_copy(w_t[:, cb * 9 + k, :], ptr)

            for b in range(1, B_LOCAL):
                xin, xp = alloc_sample()
                load_chunk(xin, xp, b, 0, H)
                xps.append(xp)

            o_v = o_d.rearrange("b (cb cp) h w -> b cb cp (h w)", cb=2)
            # Sample 0's tiles are ordered so the first half only needs
            # x-chunk0 (top rows) and cb=1 weights have time to land.
            sample0_order = (
                [(0, t) for t in range(N_T // 2)]
                + [(1, t) for t in range(N_T // 2)]
                + [(0, t) for t in range(N_T // 2, N_T)]
                + [(1, t) for t in range(N_T // 2, N_T)]
            )
            std_order = [(cb, t) for cb in range(2) for t in range(N_T)]
            for b in range(B_LOCAL):
                xp = xps[b]
                for cb, t in std_order:
                        h0 = t * ROWS
                        ps = pspool.tile([128, ROWS * W], F32)
                        for k in range(9):
                            kh, kw = divmod(k, 3)
                            rhs = xp[:, h0 + kh : h0 + kh + ROWS, kw : kw + W]
                            nc.tensor.matmul(
                                ps,
                                w_t[:, cb * 9 + k, :],
                                rhs,
                                start=(k == 0),
                                stop=(k == 8),
                            )
                        ob = opool.tile([128, ROWS * W], F32)
                        nc.vector.tensor_scalar_add(ob, ps, bias_sb[:, cb : cb + 1])
                        nc.sync.dma_start(
                            o_v[b, cb, :, h0 * W : (h0 + ROWS) * W], ob
                        )

    nc.finalize()
    return nc


def run(x: np.ndarray, weight: np.ndarray, bias: np.ndarray, **spmd_kwargs):
    x = np.ascontiguousarray(x, dtype=np.float32)
    weight = np.ascontiguousarray(weight, dtype=np.float32)
    bias = np.ascontiguousarray(bias, dtype=np.float32)

    nc = build_nc()
    in_maps = [
        {
            "x": x[c * B_LOCAL : (c + 1) * B_LOCAL],
            "weight": weight,
            "bias": bias,
        }
        for c in range(N_CORES)
    ]
    res = run_bass_kernel_spmd(
        nc, in_maps, core_ids=list(range(N_CORES)), **spmd_kwargs
    )
    out = np.concatenate([r["out"] for r in res.results], axis=0)
    return out, res


def kernel(x: np.ndarray, weight: np.ndarray, bias: np.ndarray) -> np.ndarray:
    out, _ = run(x, weight, bias)
    return out

